# revision 32
# baseline (speedup 1.0000x reference)
import sys

for _p in ("/opt/trn_rl_repo", "/opt/trn_rl_repo/concourse"):
    if _p not in sys.path:
        sys.path.insert(0, _p)

import numpy as np
import ml_dtypes

N_CORES = 8
B, H, W_DIM, C = 8, 32, 32, 288
NP = H * W_DIM         # 1024 points per image
N = 4 * NP             # 4096 points per core (one image QUAD per core)
O = 64                 # codewords total
OL = 16                # codewords per core (o-quarter sharding)
CHUNK = 512            # PSUM bank free size (fp32)
NCH = N // CHUNK       # 8 chunks
# Core c handles image pair (2p, 2p+1), p = c % 4, and codeword half
# h = c // 4 (o in [32h, 32h+32)). Doubling the free dim to 2048 amortizes
# the fixed per-op overheads (~280ns Act, ~250ns DVE pair) over 2x columns.

# Per-engine cost (us) of one [<=128, 2048] abs-production op, extrapolated
# from HW-measured 1024-col costs: Act = 2048*0.833ns + ~0.3us; DVE =
# TS-sub (2x) + STT abs->fp8 (1x) = 2048*1.562ns + ~0.25us. GPSIMD excluded
# (its software TensorScalar is ~15us/op and poisons co-running DVE).
COST_ACT = 3.71
COST_DVE = 6.65

_CACHE = {}


def _patch_drain_split():
    # The end-of-TileContext drain waits on the FULL global clock (engines +
    # one sem per DMA HW queue), overflowing the CTRL_NO struct's sync-wait
    # slots in walrus. Split: emit one 1-wait SP nop per clock component
    # first; the original drain's full-clock add_sem_waits then elides
    # everything via SP wait history.
    import concourse.tile as tile_mod
    from concourse.vector_clock import ScopedClock, VectorClock

    if getattr(tile_mod.TileContext, "_drain_split_patched", False):
        return

    def _drain_and_barrier(self, tick_clock, wait_clock):
        gc = tick_clock.global_clock
        for idx in range(len(gc)):
            tick = gc[idx]
            if tick <= 0:
                continue
            nop = self.nc.sync.nop(nofuse=True, hint="drain_split")
            vc = VectorClock()
            vc.require_at_least(idx, tick)
            wait_clock.add_sem_waits(nop.ins, ScopedClock({None: vc}))
        # Waitless drain: the nops above (same SP sequencer, in order)
        # already guarantee every sem is at its final value here.
        self.nc.sync.drain()
        self.nc.all_engine_barrier()
        assert self.sems is not None
        popped = self.nc._tile_sem_poison_stack.pop()
        assert popped is self._sem_poison
        self.nc.clear_and_free_semaphores(list(self.sems.allocated().values()))
        self.nc.all_engine_barrier()

    tile_mod.TileContext._drain_and_barrier = _drain_and_barrier
    tile_mod.TileContext._drain_split_patched = True


def _assign_units():
    """Static engine assignment for the 36 production units (32 full-o units
    + 4 tail-pair units), each two [128, 2048] abs ops, greedily balancing
    projected per-engine finish time."""
    units = ([("full", o) for o in range(4)]
             + [("tail", gp) for gp in range(2)]
             + [("full", o) for o in range(4, OL)])
    # head-starts: DVE does zdr memsets + absorbers + 2 end extractions; Act
    # does its absorber + table load + 2 end extractions.
    t = {"act": 2.6, "dve": 0.6}
    cost = {"act": 2 * COST_ACT, "dve": 2 * COST_DVE}
    out = []
    for u in units:
        e = min(t, key=lambda k: t[k] + cost[k])
        t[e] += cost[e]
        out.append((u, e))
    return out


def _build_program():
    import concourse.bass as bass
    import concourse.tile as tile
    from concourse import mybir

    _patch_drain_split()
    nc = bass.Bass("TRN2", debug=False, num_devices=N_CORES)

    f32 = mybir.dt.float32
    bf16 = mybir.dt.bfloat16
    fp8 = mybir.dt.float8e4
    Abs = mybir.ActivationFunctionType.Abs
    Ident = mybir.ActivationFunctionType.Identity
    DR = mybir.MatmulPerfMode.DoubleRow
    AOP = mybir.AluOpType

    # x transposed per core (2 images side by side): rows = channel, cols =
    # point. xa/xb are channel blocks 0:128 / 128:256; xt is channels
    # 256:288 replicated to all four SBUF quadrants so one op covers the
    # channel tail of four codewords.
    xa_d = nc.dram_tensor("xa", [128, N], bf16, kind="ExternalInput")
    xb_d = nc.dram_tensor("xb", [128, N], bf16, kind="ExternalInput")
    xt_d = nc.dram_tensor("xt", [128, N], bf16, kind="ExternalInput")
    # wcst cols (per o-half): 0:64 = -w for c-blocks (i*32+o), 64:72 =
    # quadrant-packed tail -w[256+j, 4g+q] at [32q+j, 64+g], 72:144 = +w
    # (same layout), col 144 = bias b (rows 0:32).
    WC = 2 * OL + 8
    wcst_d = nc.dram_tensor("wcst", [128, 2 * WC + 1], f32, kind="ExternalInput")
    # tail routing one-hots: [32q+j, i, 32*gp + 4*(2gp+i)+q] = +1
    ztail_d = nc.dram_tensor("ztail", [128, 2, 2 * OL], fp8, kind="ExternalInput")
    out_d = nc.dram_tensor("out_t", [OL, N], f32, kind="ExternalOutput")

    xa, xb, xt = xa_d.ap(), xb_d.ap(), xt_d.ap()
    wcst, ztail_a, out_t = wcst_d.ap(), ztail_d.ap(), out_d.ap()

    from contextlib import ExitStack

    with tile.TileContext(nc) as tc, ExitStack() as ctx:
        const_pool = ctx.enter_context(tc.tile_pool(name="const", bufs=1))
        # One fresh buffer per production unit (36 x 512KB = 18MB SBUF):
        # reusing buffers would add WAW/WAR sem waits on the producing ops,
        # overflowing walrus's single sync-wait slot per instruction.
        prod_pool = ctx.enter_context(tc.tile_pool(name="prod", bufs=18))
        tmp_pool = ctx.enter_context(tc.tile_pool(name="tmp", bufs=1))
        psum_pool = ctx.enter_context(tc.tile_pool(name="ps", bufs=1, space="PSUM"))

        # --- SBUF constants (wcst first: absorbers + act table load chain
        # off it while the bigger x transfers stream in) ----------------------
        wcst_sb = const_pool.tile([128, 2 * WC + 1], f32, name="wcst_sb")
        nc.sync.dma_start(wcst_sb[:], wcst[:, :])
        xa_sb = const_pool.tile([128, N], bf16, name="xa_sb")
        nc.sync.dma_start(xa_sb[:], xa[:, :])
        xb_sb = const_pool.tile([128, N], bf16, name="xb_sb")
        nc.sync.dma_start(xb_sb[:], xb[:, :])
        xt_sb = const_pool.tile([128, N], bf16, name="xt_sb")
        nc.sync.dma_start(xt_sb[:], xt[:, :])
        ztail_sb = const_pool.tile([128, 2, 2 * OL], fp8, name="ztail_sb")
        nc.sync.dma_start(ztail_sb[:], ztail_a[:, :, :])
        negw_sb = wcst_sb[:, 0:WC]
        wbf_sb = wcst_sb[:, WC : 2 * WC]
        b_sb = wcst_sb[:, 2 * WC : 2 * WC + 1]

        # Full-pair routing: all-ones column at absolute col 31 (both k-subs);
        # lhsT slice [:, :, 31-o : 63-o] puts the hot column at local index o.
        zdr = const_pool.tile([128, 2, 32], fp8, name="zdr")
        nc.vector.memset(zdr[:], 0.0)
        nc.vector.memset(zdr[:, :, 15:16], 1.0)

        # --- walrus 1-sync-wait discipline: per-engine absorber ops ---------
        scr_d = const_pool.tile([1, 16], f32, name="scr_d")
        scr_a = const_pool.tile([1, 16], f32, name="scr_a")

        # DVE production TS ops carry a recurring self-WAR wait (tmp_d), so
        # they cannot also absorb a fresh DMA component: pre-absorb every
        # tensor the DVE stream reads. Act ops are wait-free per-op and only
        # need wcst (the first op's second component) pre-absorbed.
        for k, s in enumerate((wcst_sb, xa_sb, xb_sb, xt_sb)):
            nc.vector.tensor_scalar_add(scr_d[0:1, k : k + 1], s[0:1, 0:1], 0.0)
        nc.scalar.activation(scr_a[0:1, 0:1], wcst_sb[0:1, 0:1], Abs,
                             bias=wcst_sb[0:1, 0:1])

        tmp_d = tmp_pool.tile([128, N], bf16, name="tmp_d", tag="tmp_d")

        # --- PSUM banks: one accumulation group per 512-col chunk -----------
        bank = [psum_pool.tile([128, CHUNK], f32, name=f"bank{ch}")
                for ch in range(NCH)]

        # PE absorbers: load the DVE (zdr memset) and ztail-DMA sems into PE
        # wait history via singleton matmuls before the real DR stream. All 8
        # banks hold chunks; the absorbers write partition 64 of bank 0,
        # disjoint from the real groups' rows 0:16 (regions are per-partition
        # byte ranges).
        nc.tensor.matmul(bank[0][64:65, 0:1], lhsT=zdr[:, 0, 0:1],
                         rhs=zdr[:, 0, 0:1], start=True, stop=True)
        nc.tensor.matmul(bank[0][64:65, 0:1], lhsT=ztail_sb[:, 0, 0:1],
                         rhs=ztail_sb[:, 0, 0:1], start=True, stop=True)

        def produce(eng, dst, src, col):
            if eng == "act":
                nc.scalar.activation(dst, src, Abs,
                                     bias=negw_sb[:, col : col + 1])
            else:
                nc.vector.tensor_scalar_sub(tmp_d[:], src,
                                            wbf_sb[:, col : col + 1])
                nc.vector.scalar_tensor_tensor(dst, tmp_d[:], -1.0, tmp_d[:],
                                               op0=AOP.mult, op1=AOP.max)

        assignment = _assign_units()
        n_units = len(assignment)
        done = [0]

        for (kind, a), eng in assignment:
            dt = prod_pool.tile([128, 2, N], fp8, name="dt", tag="u")
            if kind == "full":
                o = a
                for i, src in enumerate((xa_sb, xb_sb)):
                    produce(eng, dt[:, i, :], src, i * OL + o)
                lhsT = zdr[:, :, 15 - o : 31 - o]
            else:
                gp = a
                for i in range(2):
                    produce(eng, dt[:, i, :], xt_sb, 2 * OL + 2 * gp + i)
                lhsT = ztail_sb[:, :, OL * gp : OL * (gp + 1)]
            for ch in range(NCH):
                nc.tensor.matmul(
                    bank[ch][0:OL, :],
                    lhsT=lhsT,
                    rhs=dt[:, :, CHUNK * ch : CHUNK * (ch + 1)],
                    start=(done[0] == 0),
                    stop=(done[0] == n_units - 1),
                    perf_mode=DR,
                )
            done[0] += 1

        # --- extraction: out[:, ch] = bank[ch] + b, one op per chunk --------
        # chunks 0,1 on DVE and 2,3 on Act run in parallel; each carries only
        # its bank's PE stop wait; each engine's half DMAs out independently.
        out_sb = const_pool.tile([OL, N], f32, name="out_sb")
        for ch in (0, 1, 2, 3):
            nc.vector.tensor_scalar_add(
                out_sb[0:OL, CHUNK * ch : CHUNK * (ch + 1)],
                bank[ch][0:OL, :], b_sb[0:OL, 0:1])
        nc.sync.dma_start(out_t[:, 0 : 4 * CHUNK], out_sb[0:OL, 0 : 4 * CHUNK])
        for ch in (4, 5, 6, 7):
            nc.scalar.activation(
                out_sb[0:OL, CHUNK * ch : CHUNK * (ch + 1)],
                bank[ch][0:OL, :], Ident, bias=b_sb[0:OL, 0:1])
        nc.sync.dma_start(out_t[:, 4 * CHUNK : 8 * CHUNK],
                          out_sb[0:OL, 4 * CHUNK : 8 * CHUNK])

    return nc


def _prep_inputs(x, w, b):
    xs = x.reshape(B, NP, C).astype(np.float32)
    w = np.asarray(w, dtype=np.float32)
    b = np.asarray(b, dtype=np.float32)
    fp8 = ml_dtypes.float8_e4m3
    bf16 = ml_dtypes.bfloat16
    WC = 2 * OL + 8

    wcsts = []
    for h in range(4):
        wh = w[:, OL * h : OL * (h + 1)]  # [288, 16]
        negw = np.zeros((128, WC), dtype=np.float32)
        for i in range(2):
            negw[:, i * OL : (i + 1) * OL] = -wh[128 * i : 128 * (i + 1), :]
        for g in range(OL // 4):
            for q in range(4):
                negw[32 * q : 32 * q + 32, 2 * OL + g] = -wh[256:288, 4 * g + q]
        wcst = np.zeros((128, 2 * WC + 1), dtype=np.float32)
        wcst[:, 0:WC] = negw
        wcst[:, WC : 2 * WC] = -negw
        wcst[0:OL, 2 * WC] = b[OL * h : OL * (h + 1)]
        wcsts.append(wcst)

    ztail = np.zeros((128, 2, 2 * OL), dtype=np.float32)
    for gp in range(2):
        for i in range(2):
            for q in range(4):
                o = 4 * (2 * gp + i) + q
                ztail[32 * q : 32 * q + 32, i, OL * gp + o] = 1.0
    ztail = ztail.astype(fp8)

    in_maps = []
    for core in range(N_CORES):
        p, h = core % 2, core // 2
        xT = np.concatenate([xs[4 * p + k].T for k in range(4)], axis=1)  # [C, 4096]
        in_maps.append({
            "xa": xT[0:128].astype(bf16),
            "xb": xT[128:256].astype(bf16),
            "xt": np.tile(xT[256:288], (4, 1)).astype(bf16),
            "wcst": wcsts[h], "ztail": ztail,
        })
    return in_maps


def kernel(x, w, b):
    from concourse.bass_utils import run_bass_kernel_spmd

    if "nc" not in _CACHE:
        _CACHE["nc"] = _build_program()
    nc = _CACHE["nc"]

    in_maps = _prep_inputs(x, w, b)
    res = run_bass_kernel_spmd(nc, in_maps, list(range(N_CORES)))
    out = np.empty((B, NP, O), dtype=np.float32)
    for core in range(N_CORES):
        p, h = core % 2, core // 2
        r = np.asarray(res.results[core]["out_t"], dtype=np.float32)  # [OL, 4096]
        for k in range(4):
            out[4 * p + k, :, OL * h : OL * (h + 1)] = r[:, k * NP : (k + 1) * NP].T
    return out
